# revision 7
# baseline (speedup 1.0000x reference)
"""Trainium2 Bass kernel for the parameterized-quantum-circuit policy network.

Math: the circuit is psi = V5 E4 V4 ... E0 V0 e0 where V_l are x-independent
1024x1024 unitaries (single-qubit rotations + CZ ring, all built from theta)
and E_l(x) = tensor-prod of Rx(lam*x). Using Rx = H Rz H (H = Hadamard^{ox10}),
E_l = H D_l(x) H with D_l diagonal. Folding the H's into the V's:

    psi = W5 D4 W4 D3 W3 D2 W2 D1 W1 (D0 * psi1)

with W_l = H V_l H (l=1..4), W5 = V5 H, psi1 = first column of H V0, and
D_l[b,k] = exp(-i * phi), phi = sum_q (1-2 bits[k,q]) * lam[l,q] * x[b,q] / 2.

Device work per core (batch-sharded 2048 -> 8 x 256, state [1024, 256] fp16
with dim on partitions). The engine-limiting resources are VectorE and PE
roughly in balance (measured PE rate ~50 ns per 128x128x256 fp16 matmul;
DVE at the 2.3x-errata rate), so the complex 1024x1024 matmul per layer is
done two ways:
 - layers 1..4: Karatsuba (k1=C@a, k2=D@b, k3=S@s with S=C+D, s=a+b;
   re=k1-k2, im=k3-k1-k2) with fp8e4 weights x8 (fp8 stationary runs at
   fp16 speed and keeps weight DMA at 0.75 MB/pass; the x8 scale uses the
   fp8 normal range and is divided out for free in the ACT staging copies'
   scale field). 48 matmuls/pass.
 - layer 5: the 4-matmul form with both recombinations free in PSUM
   accumulation (psum_re += C@a, += D@(-b); psum_im += D@a, += C@b) with
   fp16 weights: full precision where it matters most and no k-recombine
   vector ops. The layer-4 evacuation emits the -b plane this needs.
State carries three fp16 planes: (a, b, s=a+b) into layers 1..4 and
(a, b, -b) into layer 5. Phases: one K=12 fp16 matmul per k-tile emits
[phi | phi+0.25] into a PSUM bank (doubled-column xt with a [zeros|ones]
row gates the +0.25, which doubles as the pi/2 cosine bias); range
reduction is a single DVE tensor_scalar (y = phi mod 1 - 0.5, floor-mod)
and ACT evaluates sin(-2pi*y) = sin(2pi*phi) with the LUT argument kept in
(-pi, pi]; both sin and cos come from one wide ACT Sin. gpsimd takes two
to three SBUF-only fp16 ops per pass (real-HW gpsimd runs far below the
cost model's rate). Readout sum(|psi|^2 * Zsign) via M=1 reduce matmuls,
sigmoid for the 2-way softmax. All theta/lam/w-derived tables are host-
precomputed; all x-dependent compute runs on device.
"""

import sys

sys.path.insert(0, "/opt/trn_rl_repo")

import numpy as np
import ml_dtypes
import concourse.bass as bass
import concourse.mybir as mybir
import concourse.tile as tile
from concourse.bass_utils import run_bass_kernel_spmd

F32 = mybir.dt.float32
F16 = mybir.dt.float16
F8 = mybir.dt.float8e4
NP8 = ml_dtypes.float8_e4m3
AF = mybir.ActivationFunctionType
ALU = mybir.AluOpType

NQ = 10
DIM = 1024
L = 5
B = 2048
NC = 8
BC = B // NC  # 256 batch per core
KT = DIM // 128  # 8 k tiles
BETA = 1.0
NKARA = 4  # layers 1..NKARA use fp8 Karatsuba; later layers fp16 reim
WS = 8.0  # fp8 weight scale (divided out in the ACT staging copies)

PI = float(np.pi)
MAGIC = float(1.5 * 2**23)
TWOPI = float(2.0 * np.pi)


# ---------------------------------------------------------------- host math
_bits = (np.arange(DIM)[:, None] >> (NQ - 1 - np.arange(NQ))) & 1
_SIGNS = (1.0 - 2.0 * _bits).astype(np.float64)
_cz = np.ones(DIM)
for _i in range(NQ):
    _cz *= 1.0 - 2.0 * (_bits[:, _i] * _bits[:, (_i + 1) % NQ])
_ZSIGN = (1.0 - 2.0 * (_bits.sum(1) % 2)).astype(np.float64)


def _rx(t):
    c, s = np.cos(0.5 * t), np.sin(0.5 * t)
    return np.array([[c, -1j * s], [-1j * s, c]])


def _ry(t):
    c, s = np.cos(0.5 * t), np.sin(0.5 * t)
    return np.array([[c, -s], [s, c]])


def _rz(t):
    e = np.exp(-0.5j * t)
    return np.array([[e, 0.0], [0.0, np.conj(e)]])


def _build_weights(theta, lam):
    th = np.asarray(theta, np.float64).reshape(L + 1, NQ, 3)
    lm = np.asarray(lam, np.float64).reshape(L, NQ)
    H1 = np.array([[1.0, 1.0], [1.0, -1.0]]) / np.sqrt(2.0)
    H = np.array([[1.0]])
    for _ in range(NQ):
        H = np.kron(H, H1)
    V = []
    for l in range(L + 1):
        U = np.array([[1.0]], dtype=np.complex128)
        for q in range(NQ):
            U = np.kron(U, _rz(th[l, q, 2]) @ _ry(th[l, q, 1]) @ _rx(th[l, q, 0]))
        V.append(_cz[:, None] * U)
    psi1 = (H @ V[0])[:, 0]
    W = [H @ V[l] @ H for l in range(1, L)] + [V[L] @ H]
    A = np.empty((L, NQ, DIM))
    for l in range(L):
        A[l] = (_SIGNS * (lm[l] / 2.0)).T
    return W, psi1, A


# ---------------------------------------------------------------- device IR
def _legalize_single_wait(nc):
    """This walrus build accepts only one sync-wait per instruction: hoist
    extra waits onto injected single-wait EventSemaphore carriers."""
    n_fix = 0
    for f in nc.m.functions:
        for bb in f.blocks:
            insts = bb.instructions
            new = []
            for ins in insts:
                si = ins.sync_info
                if si is not None and len(si.on_wait) > 1:
                    for w in si.on_wait[:-1]:
                        n_fix += 1
                        ev = mybir.InstEventSemaphore(
                            name=f"waitfix_{ins.name}_{n_fix}", ins=[], outs=[]
                        )
                        ev.engine = ins.engine
                        ev.sync_info = mybir.SyncInfo(on_wait=[w], on_update=[])
                        new.append(ev)
                    ins.sync_info = mybir.SyncInfo(
                        on_wait=[si.on_wait[-1]], on_update=si.on_update
                    )
                new.append(ins)
            insts[:] = new
    return n_fix


def _build_nc(mm_f32r=False, debug=False, repeat=1, internal_weights=False):
    nc = bass.Bass()
    wkind = "Internal" if internal_weights else "ExternalInput"

    # xt rows 0..9 = [x.T | x.T]; row 10 = ones; row 11 = [zeros | ones]
    # (row 11 activates the +0.25 cosine branch only in the second half)
    xt_d = nc.dram_tensor("xt", [NQ + 3, 2 * BC], F16, kind="ExternalInput")
    # at rows 0..9 = A/2pi, row 10 = -angle(psi1)/2pi (l=0 only; folds the
    # initial-state phase into the l=0 phase tables), row 11 = 0.25
    at_d = nc.dram_tensor("at", [NQ + 3, L, DIM], F16, kind="ExternalInput")
    psire_d = nc.dram_tensor("psire", [128, KT], F32, kind="ExternalInput")
    psiim_d = nc.dram_tensor("psiim", [128, KT], F32, kind="ExternalInput")
    zs_d = nc.dram_tensor("zs", [128, KT], F16, kind="ExternalInput")
    wsc_d = nc.dram_tensor("wsc", [1, 1], F32, kind="ExternalInput")
    wall_d = {}
    for l in range(1, L + 1):
        if l <= NKARA:
            # [pass, 128, k-tile, C(2mi x 128) | D | S] fp8, x WS
            wall_d[l] = nc.dram_tensor(f"wall{l}", [4, 128, KT, 768], F8, kind=wkind)
        else:
            # [pass, 128, k-tile, C(2mi x 128) | D] fp16
            wall_d[l] = nc.dram_tensor(f"wall{l}", [4, 128, KT, 512], F16, kind=wkind)
    probs_d = nc.dram_tensor("probs", [2, BC], F32, kind="ExternalOutput")
    if debug:
        dbga_d = nc.dram_tensor("dbga", [L + 1, 128, KT, BC], F16, kind="ExternalOutput")
        dbgb_d = nc.dram_tensor("dbgb", [L + 1, 128, KT, BC], F16, kind="ExternalOutput")

    with tile.TileContext(nc) as tc:
        with (
            tc.tile_pool(name="consts", bufs=1) as cpool,
            tc.tile_pool(name="state", bufs=3) as spool,
            tc.tile_pool(name="wts", bufs=4) as wpool,
            tc.tile_pool(name="trig", bufs=3) as tpool,
            tc.tile_pool(name="scr", bufs=6) as upool,
            tc.tile_pool(name="outp", bufs=1) as opool,
            tc.tile_pool(name="psum", bufs=1, space="PSUM") as ppool,
        ):
            # ---- constants
            xt_t = cpool.tile([NQ + 3, 2 * BC], F16)
            nc.sync.dma_start(xt_t[:], xt_d[:])
            at_t = cpool.tile([NQ + 3, L, DIM], F16)
            nc.sync.dma_start(at_t[:], at_d[:])
            psire_t = cpool.tile([128, KT], F32)
            nc.sync.dma_start(psire_t[:], psire_d[:])
            psiim_t = cpool.tile([128, KT], F32)
            nc.sync.dma_start(psiim_t[:], psiim_d[:])
            zs_t = cpool.tile([128, KT], F16)
            nc.sync.dma_start(zs_t[:], zs_d[:])
            wsc_t = cpool.tile([1, 1], F32)
            nc.sync.dma_start(wsc_t[:], wsc_d[:])
            zbias = cpool.tile([128, 1], F32)
            nc.vector.memset(zbias[:], 0.0)
            nmag_b = cpool.tile([128, 1], F32)
            nc.vector.memset(nmag_b[:], -MAGIC)
            zb1 = cpool.tile([1, 1], F32)
            nc.vector.memset(zb1[:], 0.0)

            def phase_tile(l, cs_t, t):
                """cs_t[:, t] [128, 2, BC] fp16 <- sin (slot 0) / cos (slot
                1) of 2pi*phi' for k-tile t of layer l. phi' = phi/2pi from
                the PE (A tables pre-divided by 2pi); phi'+0.25 from the
                11th ones-row, and sin of that column is cos(2pi*phi').
                Range reduction: y = (phi' mod 1) - 0.5 in [-0.5, 0.5)
                (floor-mod, one DVE op), then sin(-2pi*y) == sin(2pi*phi')
                keeps the Sin LUT argument inside (-pi, pi]."""
                ph = ppool.tile([128, 2, BC], F32, tag="ph", name="ph", bufs=2)
                nc.tensor.matmul(
                    ph[:],
                    at_t[:, l, 128 * t : 128 * (t + 1)],
                    xt_t[:],
                    start=True,
                    stop=True,
                    skip_group_check=True,
                )
                # ACT does the rint: n' = -(MAGIC + rint(ph)); then one
                # DVE op recovers the fraction fr = (n' + MAGIC) + ph
                n12 = upool.tile([128, 2, BC], F32, tag="rn1", name="n12", bufs=2)
                nc.scalar.activation(
                    n12[:], ph[:], AF.Identity, bias=nmag_b[:], scale=-1.0
                )
                fr12 = upool.tile([128, 2, BC], F32, tag="rf1", name="fr12", bufs=2)
                nc.vector.scalar_tensor_tensor(
                    fr12[:], n12[:], MAGIC, ph[:], ALU.add, ALU.add
                )
                nc.scalar.activation(
                    cs_t[:, t, :, :], fr12[:], AF.Sin, bias=zbias[:], scale=TWOPI
                )

            def emit_round(dump_debug, cs_pre):
                # ---- init: state = D_0 * psi1, planes (a, b, s) fp16
                if cs_pre is None:
                    cs = {0: tpool.tile([128, KT, 2, BC], F16, tag="cs", name="cs0")}
                    for t in range(KT):
                        phase_tile(0, cs[0], t)
                else:
                    cs = dict(cs_pre)
                st_t = spool.tile([128, KT, 3, BC], F16, tag="st", name="st0")
                for t in range(KT):
                    # psi1's phase is folded into the l=0 tables, so
                    # state0 = |psi1| * (cos - i sin)
                    nc.vector.tensor_scalar_mul(
                        st_t[:, t, 0, :], cs[0][:, t, 1, :], psire_t[:, t : t + 1]
                    )
                    nc.vector.tensor_scalar_mul(
                        st_t[:, t, 1, :], cs[0][:, t, 0, :], psiim_t[:, t : t + 1]
                    )
                    nc.vector.tensor_add(
                        st_t[:, t, 2, :], st_t[:, t, 0, :], st_t[:, t, 1, :]
                    )
                if cs_pre is None:
                    cs[1] = tpool.tile([128, KT, 2, BC], F16, tag="cs", name="cs1")
                    for t in range(KT):
                        phase_tile(1, cs[1], t)
                csn = {}

                # ---- layers
                for l in range(1, L + 1):
                    kara = l <= NKARA
                    if l < L:
                        csl = cs[l]
                        st2_t = spool.tile([128, KT, 3, BC], F16, tag="st", name="st2")
                    else:
                        sq_t = spool.tile([128, KT, BC], F16, tag="sq", name="sq")
                    if l + 1 < L:
                        # phases for layer l+1, computed in layer-l pass gaps
                        fill_l = l + 1
                        fill_cs = cs[l + 1] = tpool.tile(
                            [128, KT, 2, BC], F16, tag="cs", name=f"cs{l + 1}"
                        )
                    else:
                        # layers 4/5 gaps prefetch next round's l=0/1 phases
                        fill_l = 0 if l == L - 1 else 1
                        fill_cs = csn[fill_l] = tpool.tile(
                            [128, KT, 2, BC], F16, tag="cs", name=f"csn{fill_l}"
                        )
                    dmas = (nc.sync, nc.scalar)
                    for pass_ in range(4):
                        # one big weight DMA per pass
                        if kara:
                            wt = wpool.tile(
                                [128, KT, 768], F8, tag="wall8", name="wt8", bufs=6
                            )
                        else:
                            wt = wpool.tile(
                                [128, KT, 512], F16, tag="wall16", name="wt16", bufs=4
                            )
                        dmas[pass_ % 2].dma_start(wt[:], wall_d[l][pass_])
                        # psum banks hold both mi halves of one pass as one
                        # accumulation group (the second half lands on
                        # start-zeroed PSUM); bufs=2 double-buffers passes
                        k1t = ppool.tile([128, 2, BC], F32, tag="k1", name="k1", bufs=2)
                        k2t = ppool.tile([128, 2, BC], F32, tag="k2", name="k2", bufs=2)
                        if kara:
                            k3t = ppool.tile([128, 2, BC], F32, tag="k3", name="k3", bufs=2)
                        for mi in range(2):
                            for k in range(KT):
                                st_ = k == 0 and mi == 0
                                sp_ = k == KT - 1 and mi == 1
                                if kara:
                                    cw = wt[:, k, 128 * mi : 128 * mi + 128]
                                    dw = wt[:, k, 256 + 128 * mi : 256 + 128 * mi + 128]
                                    sw = wt[:, k, 512 + 128 * mi : 512 + 128 * mi + 128]
                                    nc.tensor.matmul(
                                        k1t[:, mi, :], cw, st_t[:, k, 0, :],
                                        start=st_, stop=sp_, skip_group_check=True,
                                    )
                                    nc.tensor.matmul(
                                        k2t[:, mi, :], dw, st_t[:, k, 1, :],
                                        start=st_, stop=sp_, skip_group_check=True,
                                    )
                                    nc.tensor.matmul(
                                        k3t[:, mi, :], sw, st_t[:, k, 2, :],
                                        start=st_, stop=sp_, skip_group_check=True,
                                    )
                                else:
                                    cw = wt[:, k, 128 * mi : 128 * mi + 128]
                                    dw = wt[:, k, 256 + 128 * mi : 256 + 128 * mi + 128]
                                    # re = C@a - D@b ; im = D@a + C@b
                                    nc.tensor.matmul(
                                        k1t[:, mi, :], cw, st_t[:, k, 0, :],
                                        start=st_, stop=False, skip_group_check=True,
                                    )
                                    nc.tensor.matmul(
                                        k2t[:, mi, :], dw, st_t[:, k, 0, :],
                                        start=st_, stop=False, skip_group_check=True,
                                    )
                                    nc.tensor.matmul(
                                        k2t[:, mi, :], cw, st_t[:, k, 1, :],
                                        start=False, stop=sp_, skip_group_check=True,
                                    )
                                    nc.tensor.matmul(
                                        k1t[:, mi, :], dw, st_t[:, k, 2, :],
                                        start=False, stop=sp_, skip_group_check=True,
                                    )
                        # PE filler while this pass's PSUM drains
                        phase_tile(fill_l, fill_cs, 2 * pass_)
                        phase_tile(fill_l, fill_cs, 2 * pass_ + 1)
                        # ---- pair-wide evacuation + fused phase multiply
                        mg2 = slice(2 * pass_, 2 * pass_ + 2)
                        if kara:
                            # fp16 staging is safe: k1,k2,k3,re,im all share
                            # the same magnitude scale (no cancellation
                            # blowup); ACT's scale field divides out WS
                            c1 = upool.tile([128, 2, BC], F16, tag="c1", name="c1", bufs=2)
                            nc.scalar.activation(
                                c1[:], k1t[:], AF.Identity, bias=zbias[:], scale=1.0 / WS
                            )
                            c2 = upool.tile([128, 2, BC], F16, tag="c2", name="c2", bufs=2)
                            nc.scalar.activation(
                                c2[:], k2t[:], AF.Identity, bias=zbias[:], scale=1.0 / WS
                            )
                            c3 = upool.tile([128, 2, BC], F16, tag="c3", name="c3", bufs=2)
                            nc.scalar.activation(
                                c3[:], k3t[:], AF.Identity, bias=zbias[:], scale=1.0 / WS
                            )
                            re = upool.tile([128, 2, BC], F16, tag="cre", name="re", bufs=2)
                            nc.vector.tensor_sub(re[:], c1[:], c2[:])
                            t5 = upool.tile([128, 2, BC], F16, tag="t5", name="t5", bufs=2)
                            nc.gpsimd.tensor_sub(t5[:], c3[:], c1[:])
                            im = upool.tile([128, 2, BC], F16, tag="cim", name="im", bufs=2)
                            nc.vector.tensor_sub(im[:], t5[:], c2[:])
                        else:
                            re = upool.tile([128, 2, BC], F16, tag="cre", name="cre", bufs=2)
                            nc.scalar.copy(re[:], k1t[:])
                            im = upool.tile([128, 2, BC], F16, tag="cim", name="cim", bufs=2)
                            nc.scalar.copy(im[:], k2t[:])
                        if l < L:
                            stp = csl[:, mg2, 0, :]
                            ctp = csl[:, mg2, 1, :]
                            u1 = upool.tile([128, 2, BC], F16, tag="u1", name="u1", bufs=2)
                            u2 = upool.tile([128, 2, BC], F16, tag="u2", name="u2", bufs=2)
                            u3 = upool.tile([128, 2, BC], F16, tag="u3", name="u3", bufs=2)
                            u4 = upool.tile([128, 2, BC], F16, tag="u4", name="u4", bufs=2)
                            nc.vector.tensor_mul(u1[:], ctp, re[:])
                            nc.vector.tensor_mul(u2[:], stp, im[:])
                            # a' = c*re + s*im -> plane 0
                            nc.vector.tensor_add(st2_t[:, mg2, 0, :], u1[:], u2[:])
                            nc.gpsimd.tensor_mul(u3[:], ctp, im[:])
                            nc.gpsimd.tensor_mul(u4[:], stp, re[:])
                            # b' = c*im - s*re -> plane 1
                            nc.vector.tensor_sub(st2_t[:, mg2, 1, :], u3[:], u4[:])
                            if l + 1 <= NKARA:
                                # next layer is Karatsuba: plane 2 = a' + b'
                                nc.vector.tensor_add(
                                    st2_t[:, mg2, 2, :],
                                    st2_t[:, mg2, 0, :],
                                    st2_t[:, mg2, 1, :],
                                )
                            else:
                                # next layer is reim: plane 2 = -b'
                                nc.vector.tensor_scalar(
                                    st2_t[:, mg2, 2, :],
                                    st2_t[:, mg2, 1, :],
                                    -1.0, 0.0, ALU.mult, ALU.add,
                                )
                        else:
                            # |psi|^2 per element (Zsign reduce after)
                            u1 = upool.tile([128, 2, BC], F16, tag="u1", name="q1", bufs=2)
                            nc.scalar.activation(
                                u1[:], re[:], AF.Square, bias=zbias[:], scale=1.0
                            )
                            u3 = upool.tile([128, 2, BC], F16, tag="u3", name="q2", bufs=2)
                            nc.scalar.activation(
                                u3[:], im[:], AF.Square, bias=zbias[:], scale=1.0
                            )
                            nc.vector.tensor_add(sq_t[:, mg2, :], u1[:], u3[:])
                    if l < L:
                        if dump_debug:
                            nc.sync.dma_start(dbga_d[l], st2_t[:, :, 0, :])
                            nc.sync.dma_start(dbgb_d[l], st2_t[:, :, 1, :])
                        st_t = st2_t

                # ---- Zsign-weighted partition+tile reduce of |psi|^2
                ez_p = ppool.tile([1, BC], F32, tag="ph", name="ez", bufs=2)
                for mg in range(KT):
                    nc.tensor.matmul(
                        ez_p[:],
                        zs_t[:, mg : mg + 1],
                        sq_t[:, mg, :],
                        start=(mg == 0),
                        stop=(mg == KT - 1),
                        skip_group_check=True,
                    )

                # ---- readout: 2-way softmax == sigmoid of logit gap
                p0 = opool.tile([1, BC], F32, tag="p0", name="p0")
                nc.scalar.activation(
                    p0[:], ez_p[:], AF.Sigmoid, bias=zb1[:], scale=wsc_t[:, :]
                )
                p1 = opool.tile([1, BC], F32, tag="p1", name="p1")
                nc.vector.tensor_scalar(p1[:], p0[:], -1.0, 1.0, ALU.mult, ALU.add)
                nc.sync.dma_start(probs_d[0:1, :], p0[:])
                nc.sync.dma_start(probs_d[1:2, :], p1[:])
                return csn

            cs_pre = None
            for _rep in range(repeat):
                cs_pre = emit_round(debug and _rep == 0, cs_pre)

    nc.finalize()
    _legalize_single_wait(nc)
    return nc


_NC_CACHE = {}


def _get_nc(mm_f32r, debug=False, repeat=1, internal_weights=False):
    key = (bool(mm_f32r), bool(debug), int(repeat), bool(internal_weights))
    if key not in _NC_CACHE:
        _NC_CACHE[key] = _build_nc(
            mm_f32r=key[0], debug=key[1], repeat=key[2], internal_weights=key[3]
        )
    return _NC_CACHE[key]


def _make_in_maps(x, theta, lam, w):
    W, psi1, A = _build_weights(theta, lam)
    at = np.zeros((NQ + 3, L, DIM), np.float32)
    at[:NQ] = A.transpose(1, 0, 2) / (2.0 * np.pi)
    at[NQ, 0] = -np.angle(psi1) / (2.0 * np.pi)
    at[NQ + 1] = 0.25
    # row 12: +8.0 (exact in fp16) keeps phi positive so the DVE C-fmod
    # range reduction equals floor-mod; 8 is an integer so sin(2pi*.) is
    # unchanged. The phase magnitude bound below guarantees phi+8 > 0.
    at[NQ + 2] = 8.0
    pmag = np.abs(psi1)
    shared = {
        "at": np.ascontiguousarray(at).astype(np.float16),
        "psire": np.ascontiguousarray(pmag.reshape(KT, 128).T).astype(
            np.float32
        ),
        "psiim": np.ascontiguousarray(-pmag.reshape(KT, 128).T).astype(
            np.float32
        ),
        "zs": np.ascontiguousarray(_ZSIGN.reshape(KT, 128).T).astype(np.float16),
        "wsc": np.array([[BETA * (float(w[0, 0]) - float(w[0, 1]))]], np.float32),
    }
    for l in range(1, L + 1):
        WT = W[l - 1].T

        def _pack(plane):
            # [1024 c, 1024 m] -> [4pass, 128part, KT, 2mi x 128cols]
            return plane.reshape(KT, 128, 4, 256).transpose(2, 1, 0, 3)

        if l <= NKARA:
            p1 = _pack(WT.real * WS)
            p2 = _pack(WT.imag * WS)
            shared[f"wall{l}"] = np.ascontiguousarray(
                np.concatenate([p1, p2, p1 + p2], axis=3)
            ).astype(NP8)
        else:
            p1 = _pack(WT.real)
            p2 = _pack(WT.imag)
            shared[f"wall{l}"] = np.ascontiguousarray(
                np.concatenate([p1, p2], axis=3)
            ).astype(np.float16)
    x = np.asarray(x, np.float32)
    assert np.abs(at[:NQ]).sum(0).max() * np.abs(x).max() + 0.5 < 7.5, (
        "phase magnitude exceeds the +8 positivity bias"
    )
    in_maps = []
    for i in range(NC):
        m = dict(shared)
        xt = np.ones((NQ + 3, 2 * BC), np.float32)
        xc = x[BC * i : BC * (i + 1)].T
        xt[:NQ, :BC] = xc
        xt[:NQ, BC:] = xc
        xt[NQ + 1, :BC] = 0.0
        m["xt"] = np.ascontiguousarray(xt).astype(np.float16)
        in_maps.append(m)
    return in_maps


def run(x, theta, lam, w, trace=False, mm_f32r=False, debug=False, repeat=1):
    nc = _get_nc(mm_f32r, debug, repeat)
    in_maps = _make_in_maps(x, theta, lam, w)
    res = run_bass_kernel_spmd(nc, in_maps, list(range(NC)), trace=trace)
    out = np.empty((B, 2), np.float32)
    for i in range(NC):
        out[BC * i : BC * (i + 1)] = res.results[i]["probs"].T
    return out, res


def kernel(x, theta, lam, w):
    out, _ = run(x, theta, lam, w, trace=False, mm_f32r=True)
    return out


# revision 8
# speedup vs baseline: 1.1174x; 1.1174x over previous
"""Trainium2 Bass kernel for the parameterized-quantum-circuit policy network.

Math: the circuit is psi = V5 E4 V4 ... E0 V0 e0 where V_l are x-independent
1024x1024 unitaries (single-qubit rotations + CZ ring, all built from theta)
and E_l(x) = tensor-prod of Rx(lam*x). Using Rx = H Rz H (H = Hadamard^{ox10}),
E_l = H D_l(x) H with D_l diagonal. Folding the H's into the V's:

    psi = W5 D4 W4 D3 W3 D2 W2 D1 W1 (D0 * psi1)

with W_l = H V_l H (l=1..4), W5 = V5 H, psi1 = first column of H V0, and
D_l[b,k] = exp(-i * phi), phi = sum_q (1-2 bits[k,q]) * lam[l,q] * x[b,q] / 2.

Device work per core (batch-sharded 2048 -> 8 x 256, state [1024, 256] fp16
with dim on partitions): per layer one complex 1024x1024 matmul done as the
4-real-matmul form with BOTH recombinations free in PSUM accumulation:
    psum_re += C@a, psum_re += D@(-b)     (re = C@a - D@b)
    psum_im += D@a, psum_im += C@b        (im = D@a + C@b)
The state carries three fp16 planes (a, b, -b) so only the two weight
planes C=Re(W), D=Im(W) are shipped (20 MB/round, hidden under PE). This
beats Karatsuba here because the engine-limiting resource is VectorE, not
PE: Karatsuba's k1-k2 / k3-k1-k2 recombinations and its s=a+b state plane
all run on DVE, while PSUM accumulation costs zero vector ops; measured PE
rate (~50 ns per 128x128x256 fp16 matmul) makes the 4th matmul cheap.
Phases: one K=12 fp16 matmul per k-tile emits [phi | phi+0.25] into a
single PSUM bank (doubled-column xt with a [zeros|ones] row gates the
+0.25, which doubles as the pi/2 cosine bias); rint range reduction via
the 1.5*2^23 magic constant with the rounding on ACT (Identity, scale=-1,
bias=-MAGIC) and one DVE op recovering the fraction; both sin and cos come
from one wide ACT Sin. Evacuation: ACT copies re/im PSUM to fp16, DVE+
gpsimd do the phase rotation (a' = c*re + s*im, b' = c*im - s*re) writing
the three state planes (real-HW gpsimd runs far below the cost model's
rate: keep it to two SBUF-only ops per pass). Readout sum(|psi|^2 * Zsign)
via M=1 reduce matmuls, sigmoid for the 2-way softmax. All theta/lam/w-
derived tables are host-precomputed; all x-dependent compute runs on
device.
"""

import sys

sys.path.insert(0, "/opt/trn_rl_repo")

import numpy as np
import concourse.bass as bass
import concourse.mybir as mybir
import concourse.tile as tile
from concourse.bass_utils import run_bass_kernel_spmd

F32 = mybir.dt.float32
F16 = mybir.dt.float16
AF = mybir.ActivationFunctionType
ALU = mybir.AluOpType

NQ = 10
DIM = 1024
L = 5
B = 2048
NC = 8
BC = B // NC  # 256 batch per core
KT = DIM // 128  # 8 k tiles
BETA = 1.0

PI = float(np.pi)
MAGIC = float(1.5 * 2**23)
TWOPI = float(2.0 * np.pi)


# ---------------------------------------------------------------- host math
_bits = (np.arange(DIM)[:, None] >> (NQ - 1 - np.arange(NQ))) & 1
_SIGNS = (1.0 - 2.0 * _bits).astype(np.float64)
_cz = np.ones(DIM)
for _i in range(NQ):
    _cz *= 1.0 - 2.0 * (_bits[:, _i] * _bits[:, (_i + 1) % NQ])
_ZSIGN = (1.0 - 2.0 * (_bits.sum(1) % 2)).astype(np.float64)


def _rx(t):
    c, s = np.cos(0.5 * t), np.sin(0.5 * t)
    return np.array([[c, -1j * s], [-1j * s, c]])


def _ry(t):
    c, s = np.cos(0.5 * t), np.sin(0.5 * t)
    return np.array([[c, -s], [s, c]])


def _rz(t):
    e = np.exp(-0.5j * t)
    return np.array([[e, 0.0], [0.0, np.conj(e)]])


def _build_weights(theta, lam):
    th = np.asarray(theta, np.float64).reshape(L + 1, NQ, 3)
    lm = np.asarray(lam, np.float64).reshape(L, NQ)
    H1 = np.array([[1.0, 1.0], [1.0, -1.0]]) / np.sqrt(2.0)
    H = np.array([[1.0]])
    for _ in range(NQ):
        H = np.kron(H, H1)
    V = []
    for l in range(L + 1):
        U = np.array([[1.0]], dtype=np.complex128)
        for q in range(NQ):
            U = np.kron(U, _rz(th[l, q, 2]) @ _ry(th[l, q, 1]) @ _rx(th[l, q, 0]))
        V.append(_cz[:, None] * U)
    psi1 = (H @ V[0])[:, 0]
    W = [H @ V[l] @ H for l in range(1, L)] + [V[L] @ H]
    A = np.empty((L, NQ, DIM))
    for l in range(L):
        A[l] = (_SIGNS * (lm[l] / 2.0)).T
    return W, psi1, A


# ---------------------------------------------------------------- device IR
def _legalize_single_wait(nc):
    """This walrus build accepts only one sync-wait per instruction: hoist
    extra waits onto injected single-wait EventSemaphore carriers."""
    n_fix = 0
    for f in nc.m.functions:
        for bb in f.blocks:
            insts = bb.instructions
            new = []
            for ins in insts:
                si = ins.sync_info
                if si is not None and len(si.on_wait) > 1:
                    for w in si.on_wait[:-1]:
                        n_fix += 1
                        ev = mybir.InstEventSemaphore(
                            name=f"waitfix_{ins.name}_{n_fix}", ins=[], outs=[]
                        )
                        ev.engine = ins.engine
                        ev.sync_info = mybir.SyncInfo(on_wait=[w], on_update=[])
                        new.append(ev)
                    ins.sync_info = mybir.SyncInfo(
                        on_wait=[si.on_wait[-1]], on_update=si.on_update
                    )
                new.append(ins)
            insts[:] = new
    return n_fix


def _build_nc(mm_f32r=False, debug=False, repeat=1, internal_weights=False):
    nc = bass.Bass()
    wkind = "Internal" if internal_weights else "ExternalInput"

    # xt rows 0..9 = [x.T | x.T]; row 10 = ones; row 11 = [zeros | ones]
    # (row 11 activates the +0.25 cosine branch only in the second half)
    xt_d = nc.dram_tensor("xt", [NQ + 2, 2 * BC], F16, kind="ExternalInput")
    # at rows 0..9 = A/2pi, row 10 = -angle(psi1)/2pi (l=0 only; folds the
    # initial-state phase into the l=0 phase tables), row 11 = 0.25
    at_d = nc.dram_tensor("at", [NQ + 2, L, DIM], F16, kind="ExternalInput")
    psire_d = nc.dram_tensor("psire", [128, KT], F32, kind="ExternalInput")
    psiim_d = nc.dram_tensor("psiim", [128, KT], F32, kind="ExternalInput")
    zs_d = nc.dram_tensor("zs", [128, KT], F16, kind="ExternalInput")
    wsc_d = nc.dram_tensor("wsc", [1, 1], F32, kind="ExternalInput")
    wall_d = {}
    for l in range(1, L + 1):
        # [pass, 128, k-tile, C(mi0|mi1) | D(mi0|mi1)]
        wall_d[l] = nc.dram_tensor(f"wall{l}", [4, 128, KT, 512], F16, kind=wkind)
    probs_d = nc.dram_tensor("probs", [2, BC], F32, kind="ExternalOutput")
    if debug:
        dbga_d = nc.dram_tensor("dbga", [L + 1, 128, KT, BC], F16, kind="ExternalOutput")
        dbgb_d = nc.dram_tensor("dbgb", [L + 1, 128, KT, BC], F16, kind="ExternalOutput")

    with tile.TileContext(nc) as tc:
        with (
            tc.tile_pool(name="consts", bufs=1) as cpool,
            tc.tile_pool(name="state", bufs=3) as spool,
            tc.tile_pool(name="wts", bufs=4) as wpool,
            tc.tile_pool(name="trig", bufs=3) as tpool,
            tc.tile_pool(name="scr", bufs=6) as upool,
            tc.tile_pool(name="outp", bufs=1) as opool,
            tc.tile_pool(name="psum", bufs=1, space="PSUM") as ppool,
        ):
            # ---- constants
            xt_t = cpool.tile([NQ + 2, 2 * BC], F16)
            nc.sync.dma_start(xt_t[:], xt_d[:])
            at_t = cpool.tile([NQ + 2, L, DIM], F16)
            nc.sync.dma_start(at_t[:], at_d[:])
            psire_t = cpool.tile([128, KT], F32)
            nc.sync.dma_start(psire_t[:], psire_d[:])
            psiim_t = cpool.tile([128, KT], F32)
            nc.sync.dma_start(psiim_t[:], psiim_d[:])
            zs_t = cpool.tile([128, KT], F16)
            nc.sync.dma_start(zs_t[:], zs_d[:])
            wsc_t = cpool.tile([1, 1], F32)
            nc.sync.dma_start(wsc_t[:], wsc_d[:])
            zbias = cpool.tile([128, 1], F32)
            nc.vector.memset(zbias[:], 0.0)
            nmag_b = cpool.tile([128, 1], F32)
            nc.vector.memset(nmag_b[:], -MAGIC)
            zb1 = cpool.tile([1, 1], F32)
            nc.vector.memset(zb1[:], 0.0)

            def phase_tile(l, cs_t, t):
                """cs_t[:, t] [128, 2, BC] fp16 <- sin (slot 0) / cos (slot
                1) of 2pi*phi' for k-tile t of layer l. phi' = phi/2pi from
                the PE (A tables pre-divided by 2pi); phi'+0.25 from the
                11th ones-row. One MAGIC rint range reduction covers both:
                sin(2pi*(phi'+0.25 - rint(phi'+0.25))) == cos(2pi*phi'), so
                the cosine slot needs no bias and both slots share the wide
                ops. One 2KB PSUM bank holds phi (region 0) and phi+0.25
                (region 1) as a single accumulation group: start zeroes the
                bank, the second matmul lands on zeroes."""
                ph = ppool.tile([128, 2, BC], F32, tag="ph", name="ph", bufs=2)
                nc.tensor.matmul(
                    ph[:],
                    at_t[:, l, 128 * t : 128 * (t + 1)],
                    xt_t[:],
                    start=True,
                    stop=True,
                    skip_group_check=True,
                )
                # ACT does the rint: n' = -(MAGIC + rint(ph)); then one
                # DVE/gpsimd op recovers the fraction fr = (n' + MAGIC) + ph
                n12 = upool.tile([128, 2, BC], F32, tag="rn1", name="n12", bufs=2)
                nc.scalar.activation(
                    n12[:], ph[:], AF.Identity, bias=nmag_b[:], scale=-1.0
                )
                fr12 = upool.tile([128, 2, BC], F32, tag="rf1", name="fr12", bufs=2)
                nc.vector.scalar_tensor_tensor(
                    fr12[:], n12[:], MAGIC, ph[:], ALU.add, ALU.add
                )
                nc.scalar.activation(
                    cs_t[:, t, :, :], fr12[:], AF.Sin, bias=zbias[:], scale=TWOPI
                )

            def emit_round(dump_debug, cs_pre):
                # ---- init: state = D_0 * psi1, planes (a, b, -b) fp16
                if cs_pre is None:
                    cs = {0: tpool.tile([128, KT, 2, BC], F16, tag="cs", name="cs0")}
                    for t in range(KT):
                        phase_tile(0, cs[0], t)
                else:
                    cs = dict(cs_pre)
                st_t = spool.tile([128, KT, 3, BC], F16, tag="st", name="st0")
                for t in range(KT):
                    # psi1's phase is folded into the l=0 tables, so
                    # state0 = |psi1| * (cos - i sin)
                    nc.vector.tensor_scalar_mul(
                        st_t[:, t, 0, :], cs[0][:, t, 1, :], psire_t[:, t : t + 1]
                    )
                    nc.vector.tensor_scalar_mul(
                        st_t[:, t, 1, :], cs[0][:, t, 0, :], psiim_t[:, t : t + 1]
                    )
                    nc.vector.tensor_scalar_mul(
                        st_t[:, t, 2, :], cs[0][:, t, 0, :], psire_t[:, t : t + 1]
                    )
                if cs_pre is None:
                    cs[1] = tpool.tile([128, KT, 2, BC], F16, tag="cs", name="cs1")
                    for t in range(KT):
                        phase_tile(1, cs[1], t)
                csn = {}

                # ---- layers
                for l in range(1, L + 1):
                    if l < L:
                        csl = cs[l]
                        st2_t = spool.tile([128, KT, 3, BC], F16, tag="st", name="st2")
                    else:
                        sq_t = spool.tile([128, KT, BC], F16, tag="sq", name="sq")
                    if l + 1 < L:
                        # phases for layer l+1, computed in layer-l pass gaps
                        fill_l = l + 1
                        fill_cs = cs[l + 1] = tpool.tile(
                            [128, KT, 2, BC], F16, tag="cs", name=f"cs{l + 1}"
                        )
                    else:
                        # layers 4/5 gaps prefetch next round's l=0/1 phases
                        fill_l = 0 if l == L - 1 else 1
                        fill_cs = csn[fill_l] = tpool.tile(
                            [128, KT, 2, BC], F16, tag="cs", name=f"csn{fill_l}"
                        )
                    dmas = (nc.sync, nc.scalar)
                    for pass_ in range(4):
                        # one big weight DMA per pass: dram [128, k, 512]
                        # -> SBUF [128, k, 512] (few DGE issues, same bytes)
                        wt = wpool.tile(
                            [128, KT, 512], F16, tag="wall", name="wallt", bufs=6
                        )
                        dmas[pass_ % 2].dma_start(wt[:], wall_d[l][pass_])
                        # psum_re / psum_im hold both mi halves of one pass
                        # in a single bank each (one accumulation group; the
                        # second half lands on start-zeroed PSUM); bufs=2
                        # double-buffers consecutive passes
                        pre = ppool.tile([128, 2, BC], F32, tag="pre", name="pre", bufs=2)
                        pim = ppool.tile([128, 2, BC], F32, tag="pim", name="pim", bufs=2)
                        for mi in range(2):
                            for k in range(KT):
                                cw = wt[:, k, 128 * mi : 128 * mi + 128]
                                dw = wt[:, k, 256 + 128 * mi : 256 + 128 * mi + 128]
                                st_ = k == 0 and mi == 0
                                sp_ = k == KT - 1 and mi == 1
                                # re = C@a - D@b ; im = D@a + C@b
                                nc.tensor.matmul(
                                    pre[:, mi, :], cw, st_t[:, k, 0, :],
                                    start=st_, stop=False, skip_group_check=True,
                                )
                                nc.tensor.matmul(
                                    pim[:, mi, :], dw, st_t[:, k, 0, :],
                                    start=st_, stop=False, skip_group_check=True,
                                )
                                nc.tensor.matmul(
                                    pim[:, mi, :], cw, st_t[:, k, 1, :],
                                    start=False, stop=sp_, skip_group_check=True,
                                )
                                nc.tensor.matmul(
                                    pre[:, mi, :], dw, st_t[:, k, 2, :],
                                    start=False, stop=sp_, skip_group_check=True,
                                )
                        # PE filler while this pass's PSUM drains
                        phase_tile(fill_l, fill_cs, 2 * pass_)
                        phase_tile(fill_l, fill_cs, 2 * pass_ + 1)
                        # ---- pair-wide evacuation + fused phase multiply
                        mg2 = slice(2 * pass_, 2 * pass_ + 2)
                        if l < L:
                            cre = upool.tile([128, 2, BC], F16, tag="cre", name="cre", bufs=2)
                            nc.scalar.copy(cre[:], pre[:])
                            cim = upool.tile([128, 2, BC], F16, tag="cim", name="cim", bufs=2)
                            nc.scalar.copy(cim[:], pim[:])
                            stp = csl[:, mg2, 0, :]
                            ctp = csl[:, mg2, 1, :]
                            u1 = upool.tile([128, 2, BC], F16, tag="u1", name="u1", bufs=2)
                            u2 = upool.tile([128, 2, BC], F16, tag="u2", name="u2", bufs=2)
                            u3 = upool.tile([128, 2, BC], F16, tag="u3", name="u3", bufs=2)
                            u4 = upool.tile([128, 2, BC], F16, tag="u4", name="u4", bufs=2)
                            nc.vector.tensor_mul(u1[:], ctp, cre[:])
                            nc.vector.tensor_mul(u2[:], stp, cim[:])
                            # a' = c*re + s*im -> plane 0
                            nc.vector.tensor_add(st2_t[:, mg2, 0, :], u1[:], u2[:])
                            nc.gpsimd.tensor_mul(u3[:], ctp, cim[:])
                            nc.gpsimd.tensor_mul(u4[:], stp, cre[:])
                            bst = upool.tile([128, 2, BC], F16, tag="bst", name="bst", bufs=2)
                            # b' = c*im - s*re -> planes 1 (+) and 2 (-)
                            nc.vector.tensor_sub(bst[:], u3[:], u4[:])
                            nc.vector.tensor_copy(st2_t[:, mg2, 1, :], bst[:])
                            nc.vector.tensor_scalar(
                                st2_t[:, mg2, 2, :], bst[:], -1.0, 0.0, ALU.mult, ALU.add
                            )
                        else:
                            # |psi|^2 per element (Zsign reduce after)
                            u1 = upool.tile([128, 2, BC], F16, tag="u1", name="q1", bufs=2)
                            nc.scalar.activation(
                                u1[:], pre[:], AF.Square, bias=zbias[:], scale=1.0
                            )
                            u3 = upool.tile([128, 2, BC], F16, tag="u3", name="q2", bufs=2)
                            nc.scalar.activation(
                                u3[:], pim[:], AF.Square, bias=zbias[:], scale=1.0
                            )
                            nc.vector.tensor_add(sq_t[:, mg2, :], u1[:], u3[:])
                    if l < L:
                        if dump_debug:
                            nc.sync.dma_start(dbga_d[l], st2_t[:, :, 0, :])
                            nc.sync.dma_start(dbgb_d[l], st2_t[:, :, 1, :])
                        st_t = st2_t

                # ---- Zsign-weighted partition+tile reduce of |psi|^2
                ez_p = ppool.tile([1, BC], F32, tag="ph", name="ez", bufs=2)
                for mg in range(KT):
                    nc.tensor.matmul(
                        ez_p[:],
                        zs_t[:, mg : mg + 1],
                        sq_t[:, mg, :],
                        start=(mg == 0),
                        stop=(mg == KT - 1),
                        skip_group_check=True,
                    )

                # ---- readout: 2-way softmax == sigmoid of logit gap
                p0 = opool.tile([1, BC], F32, tag="p0", name="p0")
                nc.scalar.activation(
                    p0[:], ez_p[:], AF.Sigmoid, bias=zb1[:], scale=wsc_t[:, :]
                )
                p1 = opool.tile([1, BC], F32, tag="p1", name="p1")
                nc.vector.tensor_scalar(p1[:], p0[:], -1.0, 1.0, ALU.mult, ALU.add)
                nc.sync.dma_start(probs_d[0:1, :], p0[:])
                nc.sync.dma_start(probs_d[1:2, :], p1[:])
                return csn

            cs_pre = None
            for _rep in range(repeat):
                cs_pre = emit_round(debug and _rep == 0, cs_pre)

    nc.finalize()
    _legalize_single_wait(nc)
    return nc


_NC_CACHE = {}


def _get_nc(mm_f32r, debug=False, repeat=1, internal_weights=False):
    key = (bool(mm_f32r), bool(debug), int(repeat), bool(internal_weights))
    if key not in _NC_CACHE:
        _NC_CACHE[key] = _build_nc(
            mm_f32r=key[0], debug=key[1], repeat=key[2], internal_weights=key[3]
        )
    return _NC_CACHE[key]


def _make_in_maps(x, theta, lam, w):
    W, psi1, A = _build_weights(theta, lam)
    at = np.zeros((NQ + 2, L, DIM), np.float32)
    at[:NQ] = A.transpose(1, 0, 2) / (2.0 * np.pi)
    at[NQ, 0] = -np.angle(psi1) / (2.0 * np.pi)
    at[NQ + 1] = 0.25
    pmag = np.abs(psi1)
    shared = {
        "at": np.ascontiguousarray(at).astype(np.float16),
        "psire": np.ascontiguousarray(pmag.reshape(KT, 128).T).astype(
            np.float32
        ),
        "psiim": np.ascontiguousarray(-pmag.reshape(KT, 128).T).astype(
            np.float32
        ),
        "zs": np.ascontiguousarray(_ZSIGN.reshape(KT, 128).T).astype(np.float16),
        "wsc": np.array([[BETA * (float(w[0, 0]) - float(w[0, 1]))]], np.float32),
    }
    for l in range(1, L + 1):
        WT = W[l - 1].T

        def _pack(plane):
            # [1024 c, 1024 m] -> [4pass, 128part, KT, 2mi x 128cols]
            return plane.reshape(KT, 128, 4, 256).transpose(2, 1, 0, 3)

        p1 = _pack(WT.real)
        p2 = _pack(WT.imag)
        shared[f"wall{l}"] = np.ascontiguousarray(
            np.concatenate([p1, p2], axis=3)
        ).astype(np.float16)
    x = np.asarray(x, np.float32)
    in_maps = []
    for i in range(NC):
        m = dict(shared)
        xt = np.ones((NQ + 2, 2 * BC), np.float32)
        xc = x[BC * i : BC * (i + 1)].T
        xt[:NQ, :BC] = xc
        xt[:NQ, BC:] = xc
        xt[NQ + 1, :BC] = 0.0
        m["xt"] = np.ascontiguousarray(xt).astype(np.float16)
        in_maps.append(m)
    return in_maps


def run(x, theta, lam, w, trace=False, mm_f32r=False, debug=False, repeat=1):
    nc = _get_nc(mm_f32r, debug, repeat)
    in_maps = _make_in_maps(x, theta, lam, w)
    res = run_bass_kernel_spmd(nc, in_maps, list(range(NC)), trace=trace)
    out = np.empty((B, 2), np.float32)
    for i in range(NC):
        out[BC * i : BC * (i + 1)] = res.results[i]["probs"].T
    return out, res


def kernel(x, theta, lam, w):
    out, _ = run(x, theta, lam, w, trace=False, mm_f32r=True)
    return out
